# revision 21
# baseline (speedup 1.0000x reference)
"""Trainium2 Bass kernel for windowed multi-head attention (Swin-style block).

Reference computation (per batch window b of 128, N=196 tokens, C=768, H=12 heads):
    qkv  = x @ qkv_w.T + [q_bias, 0, v_bias]
    q,k,v = split(qkv);  attn = softmax(q*scale @ k.T + rel_pos_bias)
    out  = (attn @ v) @ proj_w.T + proj_b

Sharding: data-parallel over batch across 8 cores (16 windows/core).

Per-core kernel layout strategy (all matmuls consume operands in natural layout,
softmax runs in "transposed" space so no on-the-fly attention transposes):
  - x [196,768] is PE-transposed once to xT [768,196] (lhsT/rhs source).
  - Q^T,K^T [c',t] = W^T chunks (lhsT) x xT (rhs); V [t,c'] = xT (lhsT) x W^T (rhs).
  - S^T[j,i] = K^T-head (lhsT) x Q^T-head (rhs), K=64 contraction; heads pairs
    are row-tiled (partition base 0/64) so two K=64 matmuls share the PE array.
  - E^T = exp(0.125*S^T) * exp(bias)^T  (ACT exp from PSUM, DVE multiply with a
    host-precomputed exp(bias) table -- exp(a+b)=exp(a)exp(b)).
  - O^T[d,i] per head = [V-head | ones] (lhsT) x E^T (rhs): row 64 of the psum
    output is the softmax denominator for free (ones column in V).
  - per head pair: DVE copies the two denominator rows out, one reciprocal,
    one GPSIMD partition_broadcast to [64,2,196]; normalization is fused into
    the O^T PSUM->SBUF eviction (DVE multiply by the broadcast reciprocal).
  - y = O^T chunks (lhsT) x P^T (rhs) + proj_b (DVE add with broadcast bias).
Hardware notes: matmuls at different partition bases must not share a psum
bank (device-fatal); K=64 head-pair matmuls alternate PE row groups 0/64.
"""

import sys

import numpy as np

if "/opt/trn_rl_repo" not in sys.path:
    sys.path.insert(0, "/opt/trn_rl_repo")

import concourse.bass as bass  # noqa: E402
import concourse.mybir as mybir  # noqa: E402
import concourse.tile as tile  # noqa: E402
from concourse import bacc  # noqa: E402
from concourse import bass_utils  # noqa: E402
from concourse.masks import make_identity  # noqa: E402

# Problem shapes (hardcoded; kernel.py must be self-contained).
B, N, C = 128, 196, 768
H, HD = 12, 64
WS = 14
NCORES = 8
BW = B // NCORES  # 16 windows per core
NPAIRS = BW // 2
JC = 98  # j/t chunk size (2 chunks per 196-token window)
F32 = mybir.dt.float32
SCALE = HD ** -0.5  # 0.125


def _relative_position_index(ws: int) -> np.ndarray:
    coords = np.stack(np.meshgrid(np.arange(ws), np.arange(ws), indexing="ij"))
    flat = coords.reshape(2, -1)
    rel = flat[:, :, None] - flat[:, None, :]
    rel = rel.transpose(1, 2, 0).copy()
    rel[..., 0] += ws - 1
    rel[..., 1] += ws - 1
    rel[..., 0] *= 2 * ws - 1
    return rel.sum(-1)  # [N, N] int


def _build_kernel_body(ctx, tc, aps, reps=1):
    nc = tc.nc
    x_d = aps["x_sh"]
    wT_d = aps["wT"]
    pT_d = aps["pT"]
    qb_d = aps["qb"]
    vb_d = aps["vb"]
    pb_d = aps["pb"]
    eb_d = aps["expBT"]
    y_d = aps["y_sh"]

    const = ctx.enter_context(tc.tile_pool(name="const", bufs=1))

    # ---- resident constants ----
    w_sb = const.tile([128, 6, 3 * C], F32)  # W^T: [c%128, c//128, c']
    nc.sync.dma_start(out=w_sb, in_=wT_d.rearrange("(a p) m -> p a m", p=128))
    pT_sb = const.tile([128, 6, C], F32)
    nc.sync.dma_start(out=pT_sb, in_=pT_d.rearrange("(a p) m -> p a m", p=128))
    qb_sb = const.tile([128, 6], F32)
    nc.sync.dma_start(out=qb_sb, in_=qb_d.rearrange("(a p) -> p a", p=128))
    def _bcast(src, parts):
        return bass.AP(tensor=src.tensor, offset=src.offset,
                       ap=[[0, parts]] + list(src.ap))

    vb_bc = const.tile([128, C], F32)  # v_bias broadcast along partitions
    nc.sync.dma_start(out=vb_bc, in_=_bcast(vb_d, 128))
    pb_bc = const.tile([128, C], F32)
    nc.sync.dma_start(out=pb_bc, in_=_bcast(pb_d, 128))
    eb_sb = const.tile([JC, H, 2 * N], F32)  # exp(bias)^T: [j%98, h, (j//98)*196+i]
    nc.sync.dma_start(out=eb_sb, in_=eb_d.rearrange("p (h m) -> p h m", h=H))
    ident = const.tile([128, 128], F32)
    make_identity(nc, ident)

    # ---- pools ----
    xin = ctx.enter_context(tc.tile_pool(name="xin", bufs=2))
    xt = ctx.enter_context(tc.tile_pool(name="xt", bufs=2))
    qk = ctx.enter_context(tc.tile_pool(name="qk", bufs=2))
    vpool = ctx.enter_context(tc.tile_pool(name="vpool", bufs=2))
    epool = ctx.enter_context(tc.tile_pool(name="epool", bufs=2))
    opool = ctx.enter_context(tc.tile_pool(name="opool", bufs=2))
    rpool = ctx.enter_context(tc.tile_pool(name="rpool", bufs=2))
    rbc = ctx.enter_context(tc.tile_pool(name="rbc", bufs=2))
    ypool = ctx.enter_context(tc.tile_pool(name="ypool", bufs=2))
    ps_mm = ctx.enter_context(tc.tile_pool(name="ps_mm", bufs=2, space="PSUM"))
    ps_s = ctx.enter_context(tc.tile_pool(name="ps_s", bufs=2, space="PSUM"))
    ps_od = ctx.enter_context(tc.tile_pool(name="ps_od", bufs=2, space="PSUM"))

    for rep in range(reps):
      front = {}
      # software pipeline: emit pair p's GEMM front before pair p-1's
      # attention+proj so the static schedule interleaves them
      for pi in range(NPAIRS + 1):
       if pi < NPAIRS:
        wins = (2 * pi, 2 * pi + 1)
        # ---- load x ----
        xa = []
        xb = []
        for wi, w in enumerate(wins):
            ta = xin.tile([128, C], F32, tag="xa")
            nc.sync.dma_start(out=ta, in_=x_d[w, 0:128, :])
            xa.append(ta)
            tb = xin.tile([128, C], F32, tag="xb")  # rows 0:68 used
            nc.sync.dma_start(out=tb[0:68, :], in_=x_d[w, 128:196, :])
            xb.append(tb)

        # ---- transpose x -> xT [128, ck, pair-col] ----
        xT = xt.tile([128, 6, 2 * N], F32)
        for ci in range(6):
            pt = ps_mm.tile([128, 512], F32, tag="mm")
            for wi in range(2):
                nc.tensor.transpose(
                    pt[:, wi * N : wi * N + 128],
                    xa[wi][:, ci * 128 : (ci + 1) * 128],
                    ident,
                )
                nc.tensor.transpose(
                    pt[:, wi * N + 128 : wi * N + N],
                    xb[wi][0:68, ci * 128 : (ci + 1) * 128],
                    ident[0:68, 0:68],
                )
            nc.scalar.copy(out=xT[:, ci, :], in_=pt[:, 0 : 2 * N])

        # ---- Q^T / K^T chunks for the pair ----
        qk_sb = qk.tile([128, 12, 2 * N], F32)
        for cp in range(12):
            ps = ps_mm.tile([128, 512], F32, tag="mm")
            for ck in range(6):
                nc.tensor.matmul(
                    ps[:, 0 : 2 * N],
                    w_sb[:, ck, cp * 128 : (cp + 1) * 128],
                    xT[:, ck, :],
                    start=(ck == 0),
                    stop=(ck == 5),
                )
            if cp < 6:  # Q: add q_bias (per-partition scalar) on DVE
                nc.vector.tensor_scalar_add(
                    out=qk_sb[:, cp, :], in0=ps[:, 0 : 2 * N],
                    scalar1=qb_sb[:, cp : cp + 1],
                )
            else:  # K: plain copy on ACT
                nc.scalar.copy(out=qk_sb[:, cp, :], in_=ps[:, 0 : 2 * N])

        # ---- V natural (+v_bias, +ones column for denominators), per window --
        v_t = []
        for wi in range(2):
            vt = vpool.tile([128, 2, H, HD + 1], F32, tag="v")
            v_t.append(vt)
            nc.gpsimd.memset(vt[0:JC, :, :, HD : HD + 1], 1.0)
            for tck in range(2):
                for c0, nn in ((0, 512), (512, 256)):
                    ps = ps_mm.tile([128, 512], F32, tag="mm")
                    for ck in range(6):
                        nc.tensor.matmul(
                            ps[0:JC, 0:nn],
                            xT[:, ck, wi * N + tck * JC : wi * N + (tck + 1) * JC],
                            w_sb[:, ck, 2 * C + c0 : 2 * C + c0 + nn],
                            start=(ck == 0),
                            stop=(ck == 5),
                        )
                    h0 = c0 // HD
                    nh = nn // HD
                    nc.vector.tensor_add(
                        out=vt[0:JC, tck, h0 : h0 + nh, 0:HD],
                        in0=ps[0:JC, 0:nn].rearrange("p (h d) -> p h d", d=HD),
                        in1=vb_bc[0:JC, c0 : c0 + nn].rearrange(
                            "p (h d) -> p h d", d=HD),
                    )

        front[pi] = (wins, qk_sb, v_t)
       if pi >= 1:
        wins, qk_sb, v_t = front.pop(pi - 1)
        # ---- attention + proj, per window ----
        for wi, w in enumerate(wins):
            woff = wi * N
            oc = opool.tile([128, 6, N], F32, tag="oc")
            # per head-pair group: S^T, exp, bias-mul, denominators, O^T
            # (psum bank per head -- matmuls at different partition bases
            #  must not share a psum bank)
            for g in range(6):
                pss = ps_s.tile([128, 2, 512], F32, tag="s")
                for jc in range(2):
                    for hh in range(2):  # hh inner: alternate PE row groups
                        h = 2 * g + hh
                        prow = (h % 2) * 64
                        nc.tensor.matmul(
                            pss[0:JC, hh, jc * N : (jc + 1) * N],
                            qk_sb[prow : prow + 64, 6 + h // 2,
                                  woff + jc * JC : woff + (jc + 1) * JC],
                            qk_sb[prow : prow + 64, h // 2, woff : woff + N],
                            start=True,
                            stop=True,
                        )
                e2 = epool.tile([JC, 2, 2, N], F32, tag="e")  # [j, hh, jc, i]
                nc.scalar.activation(
                    out=e2,
                    in_=pss[0:JC, :, 0 : 2 * N].rearrange(
                        "p b (a n) -> p b a n", a=2),
                    func=mybir.ActivationFunctionType.Exp,
                    scale=SCALE,
                )
                nc.vector.tensor_mul(
                    e2,
                    e2,
                    eb_sb[:, 2 * g : 2 * g + 2, :].rearrange(
                        "p b (a n) -> p b a n", a=2),
                )
                # O^T (+denominator row 64, via the V ones column)
                psos = []
                dh = rpool.tile([1, 2, N], F32, tag="dh")
                for hh in range(2):
                    h = 2 * g + hh
                    pso = ps_od.tile([HD + 1, N], F32, tag="od")
                    psos.append(pso)
                    for jc in range(2):
                        nc.tensor.matmul(
                            pso,
                            v_t[wi][0:JC, jc, h, :],
                            e2[0:JC, hh, jc, :],
                            start=(jc == 0),
                            stop=(jc == 1),
                        )
                    nc.vector.tensor_copy(
                        out=dh[0:1, hh, :], in_=pso[HD : HD + 1, :])
                r2 = rpool.tile([1, 2, N], F32, tag="r")
                nc.vector.reciprocal(out=r2, in_=dh)
                rb = rbc.tile([64, 2, N], F32, tag="rb")
                nc.gpsimd.partition_broadcast(rb, r2)
                for hh in range(2):
                    h = 2 * g + hh
                    prow = (h % 2) * 64
                    nc.vector.tensor_mul(
                        oc[prow : prow + 64, h // 2, :],
                        psos[hh][0:HD, :], rb[:, hh, :])
            # proj
            for tck in range(2):
                y_t = ypool.tile([128, C], F32, tag="y")
                for c0, nn in ((0, 512), (512, 256)):
                    ps = ps_mm.tile([128, 512], F32, tag="mm")
                    for ck in range(6):
                        nc.tensor.matmul(
                            ps[0:JC, 0:nn],
                            oc[:, ck, tck * JC : (tck + 1) * JC],
                            pT_sb[:, ck, c0 : c0 + nn],
                            start=(ck == 0),
                            stop=(ck == 5),
                        )
                    nc.vector.tensor_add(
                        out=y_t[0:JC, c0 : c0 + nn],
                        in0=ps[0:JC, 0:nn],
                        in1=pb_bc[0:JC, c0 : c0 + nn],
                    )
                nc.sync.dma_start(
                    out=y_d[w, tck * JC : (tck + 1) * JC, :],
                    in_=y_t[0:JC, :],
                )


def build_program(reps=1):
    """Build + compile the per-core Bass program. Returns the Bacc instance."""
    nc = bacc.Bacc(
        "TRN2",
        target_bir_lowering=False,
        debug=False,
        enable_asserts=False,
        num_devices=NCORES,
    )
    aps = {
        "x_sh": nc.dram_tensor("x_sh", [BW, N, C], F32, kind="ExternalInput").ap(),
        "wT": nc.dram_tensor("wT", [C, 3 * C], F32, kind="ExternalInput").ap(),
        "pT": nc.dram_tensor("pT", [C, C], F32, kind="ExternalInput").ap(),
        "qb": nc.dram_tensor("qb", [C], F32, kind="ExternalInput").ap(),
        "vb": nc.dram_tensor("vb", [C], F32, kind="ExternalInput").ap(),
        "pb": nc.dram_tensor("pb", [C], F32, kind="ExternalInput").ap(),
        "expBT": nc.dram_tensor(
            "expBT", [JC, H * 2 * N], F32, kind="ExternalInput").ap(),
        "y_sh": nc.dram_tensor("y_sh", [BW, N, C], F32, kind="ExternalOutput").ap(),
    }

    from contextlib import ExitStack

    with tile.TileContext(nc) as tc:
        with ExitStack() as ctx:
            _build_kernel_body(ctx, tc, aps, reps=reps)
    nc.compile()
    return nc


_CACHED = {}


def _get_program(reps=1):
    key = f"nc{reps}"
    if key not in _CACHED:
        _CACHED[key] = build_program(reps=reps)
    return _CACHED[key]


def host_prep(qkv_w, q_bias, v_bias, rpb_table, proj_w, proj_b):
    """Host-side constant layout prep (shared across cores)."""
    idx = _relative_position_index(WS)  # [N, N] ints
    bias = rpb_table[idx.reshape(-1)].reshape(N, N, H)  # [i, j, h]
    expB = np.exp(bias.astype(np.float32))
    # expBT[r, h, jc*N + i] = expB[i, jc*JC + r, h]
    e = expB.transpose(2, 1, 0).reshape(H, 2, JC, N)  # [h, jc, r, i]
    expBT = np.ascontiguousarray(e.transpose(2, 0, 1, 3)).reshape(JC, H * 2 * N)
    return {
        "wT": np.ascontiguousarray(qkv_w.T),
        "pT": np.ascontiguousarray(proj_w.T),
        "qb": np.ascontiguousarray(q_bias),
        "vb": np.ascontiguousarray(v_bias),
        "pb": np.ascontiguousarray(proj_b),
        "expBT": expBT,
    }


def make_in_maps(x, qkv_w, q_bias, v_bias, rpb_table, proj_w, proj_b):
    shared = host_prep(qkv_w, q_bias, v_bias, rpb_table, proj_w, proj_b)
    shared = {k: np.asarray(v, np.float32) for k, v in shared.items()}
    in_maps = []
    for ci in range(NCORES):
        m = dict(shared)
        m["x_sh"] = np.ascontiguousarray(
            np.asarray(x, np.float32)[ci * BW : (ci + 1) * BW])
        in_maps.append(m)
    return in_maps


def kernel(x, qkv_w, q_bias, v_bias, rpb_table, proj_w, proj_b, _trace=False):
    """Full-input entry point: shards over 8 NeuronCores, returns full output."""
    nc = _get_program()
    in_maps = make_in_maps(x, qkv_w, q_bias, v_bias, rpb_table, proj_w, proj_b)
    res = bass_utils.run_bass_kernel_spmd(
        nc, in_maps, core_ids=list(range(NCORES)), trace=_trace)
    out = np.concatenate([res.results[i]["y_sh"] for i in range(NCORES)], axis=0)
    if _trace:
        return out, res
    return out
